# revision 3
# baseline (speedup 1.0000x reference)
"""Embedding lookup (gather) kernel for Trainium2, 8 NeuronCores.

Problem: out[i] = table[value_tensors[i]] for 212992 indices into a
[1M, 128] f32 table, reshaped to [8192, 26, 128]. (row_offsets is
arange, so the CSR segment-sum is the identity; a host-side fallback
handles the general case.)

Sharding: model-parallel by table row (range partition). The table is
split into 32 range bins of 31250 rows; core c owns bins 4c..4c+3.
The host dedupes and routes each lookup index to its owning bin, each
core gathers its rows on-device with the SWDGE dma_gather instruction,
and the host scatters the gathered rows back to the original positions
(the "all-to-all" of HugeCTR's localized embedding, at unshard time).

Perf model (from the 88us windowed baseline's trace, ntff dma slices):
  - per-engine SWDGE gather throughput is ~15-17ns per descriptor for
    256B..1KB descriptors -- the cost model's 2x penalty for <512B
    descriptors is NOT observed on HW. So window descriptors (which
    added ~28% garbage payload + ~18% padding, all of it also written
    back out) LOSE to one-descriptor-per-unique-row on both directions.
  - GpSimd (Pool) descriptor emission costs ~1.0us fixed per
    dma_gather call + ~0.34ns/desc, serial on the Pool stream. With
    896-idx calls (packet ceiling for single_packet=True) the 25-call
    emission (~30us) rivaled the DMA time. Fix: 2048-idx calls with
    single_packet=False (128 descs/ring, streamed as 2 packets) ->
    3 calls/bin, 12+warmup calls, ~22us emission.
  - calls round-robin the 4 SWDGE queues so all 16 DMA engines stay
    fed; per-bin output writes (fp16, no garbage, ~2.5% padding)
    overlap the remaining gathers on the two HWDGE rings (Sync/Scalar).
  - warmup dma_gather absorbs the ~9us cold SWDGE init during the idx
    loads; fp16 table/output halves both HBM directions (max rel err
    2^-11 vs the 2e-2 gate).

dma_gather layout (probed on HW in an earlier session): indices are
int16, wrapped over 16 partitions (ordinal i reads idx[i % 16, i // 16])
and replicated to all 8 Q7-core partition groups; gathered ordinal i
lands at dst[i % 128, i // 128]; negative idxs at the end generate no
descriptors, but each call keeps >= 16 leading non-negative idxs so all
16 engine rings still fire their completion-semaphore descriptor.
"""

import time

import numpy as np

VOCAB = 1_000_000
BATCH = 8192
SLOTS = 26
VEC = 128
NCORES = 8
NSUB = 4  # bins per core; int16 gather idx needs rows <= 32767
RSUB = VOCAB // (NCORES * NSUB)  # 31250 rows per bin
SHARD = RSUB * NSUB  # 125000 rows per core
P = 128
# Idxs per dma_gather call: 56 data descs + 1 sem desc per engine ring,
# under the 64-descriptor single-packet ceiling. single_packet=True is
# load-bearing: coalesced 56-desc ring packets process at ~14ns/desc;
# single_packet=False (measured, CH=2048) degrades to per-descriptor
# packets at ~32ns/desc -> 83.6us total vs ~21us gather-phase here.
CH = 896

LAST_RUN = None  # BassKernelResults of the most recent device run (for test.py)


def _chunks_of(N: int):
    out = []
    o = 0
    while o < N:
        out.append((o, min(CH, N - o)))
        o += CH
    return out


def _build_program(NCL: int, chunks):
    """One SPMD program for all 8 cores. NCL = padded idx slots per bin
    (multiple of 128, identical across cores/bins).

    Per core:
      shard [SHARD, VEC] fp16   - this core's 4 bins, concatenated
      idx   [P, ICOLS] i16      - [8 warm cols][bin0][bin1][bin2][bin3]
      cnt   [1, NCALL] i32      - per-gather-call runtime num_idxs
      out   [P, NSUB*W] fp16    - W = NCL cols per bin (one fp16 row
                                  per gathered ordinal, dst[i%128,i//128])
    """
    import concourse.bacc as bacc
    from concourse import mybir
    from concourse.library_config import mlp

    ncalls_bin = len(chunks)
    icols_bin = NCL // 16
    W = NCL  # output cols per bin: (NCL//128) groups of VEC
    ICOLS = 8 + NSUB * icols_bin
    NCALL = NSUB * ncalls_bin

    nc = bacc.Bacc("TRN2", num_swdge_queues=4)
    shard = nc.declare_dram_parameter(
        "shard", [SHARD, VEC], mybir.dt.float16, isOutput=False
    )
    idx = nc.declare_dram_parameter("idx", [P, ICOLS], mybir.dt.int16, isOutput=False)
    cnt = nc.declare_dram_parameter("cnt", [1, NCALL], mybir.dt.int32, isOutput=False)
    out = nc.declare_dram_parameter(
        "out", [P, NSUB * W], mybir.dt.float16, isOutput=True
    )

    sem_in = nc.alloc_semaphore("sem_in")
    sem_warm = nc.alloc_semaphore("sem_warm")
    # per-bin gather sems; the last bin's chunks are split across two
    # sems so the final write waits only on the final chunk.
    sem_g = [nc.alloc_semaphore(f"sem_g{s}") for s in range(NSUB + 1)]
    sem_out = nc.alloc_semaphore()

    idx_sb = nc.alloc_sbuf_tensor("idx_sb", [P, ICOLS], mybir.dt.int16).ap()
    cnt_sb = nc.alloc_sbuf_tensor("cnt_sb", [1, NCALL], mybir.dt.int32).ap()
    warm_out = nc.alloc_sbuf_tensor("warm_out", [P, 1, VEC], mybir.dt.float16).ap()
    g_buf = nc.alloc_sbuf_tensor("g", [P, NSUB * W], mybir.dt.float16).ap()

    nc.gpsimd.load_library(mlp)
    # cnt + warm idx cols first (threshold 32), then one idx DMA per bin
    # on the same HWDGE ring: FIFO completion => sem_in thresholds.
    nc.sync.dma_start(out=cnt_sb[:], in_=cnt[:, :]).then_inc(sem_in, 16)
    nc.sync.dma_start(out=idx_sb[:, 0:8], in_=idx[:, 0:8]).then_inc(sem_in, 16)
    for s in range(NSUB):
        a, b = 8 + s * icols_bin, 8 + (s + 1) * icols_bin
        nc.sync.dma_start(out=idx_sb[:, a:b], in_=idx[:, a:b]).then_inc(sem_in, 16)

    warm_reg = nc.gpsimd.to_reg(128)
    cregs = [nc.gpsimd.alloc_register(name=f"creg{t}") for t in range(NCALL)]

    # Warmup: one tiny gather (row 0 x128) while the idx DMAs are still
    # in flight; absorbs the ~9us cold SWDGE init (global, not
    # per-queue). Issued before reg_load so it starts ASAP.
    nc.gpsimd.wait_ge(sem_in, 32)
    nc.gpsimd.dma_gather(
        warm_out[:, :, :],
        shard[0:RSUB, :],
        idx_sb[:, 0:8],
        128,
        warm_reg,
        VEC,
        queue_num=0,
    ).then_inc(sem_warm, 16)

    # Batched loads, <= 24 regs each (52-wide measured failing to lower).
    for i in range(0, NCALL, 24):
        j = min(i + 24, NCALL)
        nc.gpsimd.reg_load(cregs[i:j], cnt_sb[0:1, i:j])

    qn = 1  # warmup used q0
    t = 0
    for s in range(NSUB):
        nc.gpsimd.wait_ge(sem_in, 32 + 16 * (s + 1))
        ibase = 8 + s * icols_bin
        for ci, (o, sz) in enumerate(chunks):
            if s < NSUB - 1:
                sem = sem_g[s]
            else:
                sem = sem_g[NSUB - 1] if ci < ncalls_bin - 1 else sem_g[NSUB]
            dst = g_buf[
                :, s * W + (o // 128) * VEC : s * W + ((o + sz) // 128) * VEC
            ].rearrange("p (k e) -> p k e", e=VEC)
            nc.gpsimd.dma_gather(
                dst,
                shard[s * RSUB : (s + 1) * RSUB, :],
                idx_sb[:, ibase + o // 16 : ibase + (o + sz) // 16],
                sz,
                cregs[t],
                VEC,
                single_packet=(sz <= 896),
                queue_num=qn % 4,
            ).then_inc(sem, 16)
            qn += 1
            t += 1
    assert t == NCALL

    # Whole-bin writes alternating between the two HWDGE rings
    # (Sync/Scalar); the last bin is split [chunks 0..n-2 | last chunk]
    # so the tail write waits only on the final gather call.
    last_o = chunks[-1][0]
    writes = []  # (engine_idx, sem, need, col0, col1)
    for s in range(NSUB - 1):
        writes.append((s % 2, sem_g[s], 16 * ncalls_bin, s * W, (s + 1) * W))
    s = NSUB - 1
    writes.append(
        (s % 2, sem_g[s], 16 * (ncalls_bin - 1), s * W, s * W + (last_o // 128) * VEC)
    )
    writes.append((NSUB % 2, sem_g[NSUB], 16, s * W + (last_o // 128) * VEC, NSUB * W))
    for ei, sem, need, c0, c1 in writes:
        eng = nc.sync if ei == 0 else nc.scalar
        eng.wait_ge(sem, need)
        eng.dma_start(out=out[:, c0:c1], in_=g_buf[:, c0:c1]).then_inc(sem_out, 16)
    nc.sync.wait_ge(sem_out, 16 * len(writes))
    nc.sync.wait_ge(sem_warm, 16)
    nc.finalize()
    return nc


def _wrap_cols(vals: np.ndarray, N: int, ecount: int) -> np.ndarray:
    """int16 idx block [16, N//16]: element i at [i%16, i//16]; slots
    [len(vals), ecount) hold 0 (valid row, gathered then ignored), slots
    [ecount, N) hold -1 (skipped by the ucode)."""
    li = np.full(N, -1, np.int16)
    li[:ecount] = 0
    li[: len(vals)] = vals.astype(np.int16)
    return li.reshape(N // 16, 16).T


def _gather_on_device(table_f16: np.ndarray, uniq: np.ndarray) -> np.ndarray:
    """emb[i] = table[uniq[i]] (fp16) computed on 8 NeuronCores."""
    global LAST_RUN
    from concourse.bass_utils import run_bass_kernel_spmd

    total = uniq.shape[0]
    nbins = NCORES * NSUB
    bin_id = (uniq // RSUB).astype(np.int32)
    local = (uniq - bin_id.astype(np.int64) * RSUB).astype(np.int32)
    counts = np.bincount(bin_id, minlength=nbins)
    assert counts.sum() == total
    bin_start = np.concatenate(([0], np.cumsum(counts)))

    NCL = max(P, ((int(counts.max()) + P - 1) // P) * P)
    chunks = _chunks_of(NCL)
    ncalls_bin = len(chunks)
    icols_bin = NCL // 16
    W = NCL

    in_maps = []
    for core in range(NCORES):
        blocks = [np.zeros((16, 8), np.int16)]  # warm cols
        cvals = []
        for s in range(NSUB):
            b = core * NSUB + s
            n = int(counts[b])
            vals = local[bin_start[b] : bin_start[b + 1]]
            o_last = chunks[-1][0]
            ecount = max(n, o_last + 16)
            blocks.append(_wrap_cols(vals, NCL, ecount))
            for o, sz in chunks:
                cvals.append(min(max(ecount - o, 0), sz))
        in_maps.append(
            {
                "shard": np.ascontiguousarray(
                    table_f16[core * SHARD : (core + 1) * SHARD]
                ),
                "idx": np.ascontiguousarray(
                    np.tile(np.concatenate(blocks, axis=1), (8, 1))
                ),
                "cnt": np.array([cvals], np.int32),
            }
        )

    # The shared device occasionally wedges transiently
    # (NRT_EXEC_UNIT_UNRECOVERABLE / profile-stop rc=-1); a fresh attempt
    # after a short pause recovers it.
    for attempt in range(3):
        try:
            nc = _build_program(NCL, chunks)
            LAST_RUN = run_bass_kernel_spmd(nc, in_maps, list(range(NCORES)))
            break
        except Exception:
            if attempt == 2:
                raise
            time.sleep(10)
    res = LAST_RUN.results

    emb = np.empty((total, VEC), np.float16)
    for core in range(NCORES):
        o = np.asarray(res[core]["out"])
        for s in range(NSUB):
            b = core * NSUB + s
            n = int(counts[b])
            if n == 0:
                continue
            reg = o[:, s * W : (s + 1) * W]
            # gathered ordinal i lands at [i % 128, (i // 128)*VEC : ...]
            rows = (
                reg.reshape(P, NCL // 128, VEC).transpose(1, 0, 2).reshape(NCL, VEC)
            )
            emb[bin_start[b] : bin_start[b + 1]] = rows[:n]
    return emb


def kernel(table, row_offsets, value_tensors, nnz_array=None, output_shape=None):
    table = np.asarray(table, dtype=np.float32)
    assert table.shape == (VOCAB, VEC)
    v = np.asarray(value_tensors).astype(np.int64).ravel()
    total = v.shape[0]

    table_f16 = table.astype(np.float16)
    uniq, inverse = np.unique(v, return_inverse=True)
    emb_u = _gather_on_device(table_f16, uniq)
    emb = emb_u[inverse].astype(np.float32)

    n_rows = BATCH * SLOTS
    ro = np.asarray(row_offsets).astype(np.int64).ravel()
    if total == n_rows and np.array_equal(ro, np.arange(total + 1)):
        return emb.reshape(BATCH, SLOTS, VEC)
    # General CSR fallback (never hit with the reference's arange offsets):
    # sum-combine values per segment on the host.
    seg = np.searchsorted(ro, np.arange(total), side="right") - 1
    combined = np.zeros((n_rows, VEC), np.float32)
    np.add.at(combined, seg, emb)
    return combined.reshape(BATCH, SLOTS, VEC)


# revision 6
# speedup vs baseline: 1.0026x; 1.0026x over previous
"""Embedding lookup (gather) kernel for Trainium2, 8 NeuronCores.

Problem: out[i] = table[value_tensors[i]] for 212992 indices into a
[1M, 128] f32 table, reshaped to [8192, 26, 128]. (row_offsets is
arange, so the CSR segment-sum is the identity; a host-side fallback
handles the general case.)

Sharding: model-parallel by table row (range partition). The table is
split into 32 range bins of 31250 rows; core c owns bins 4c..4c+3.
The host dedupes and routes each lookup index to its owning bin, each
core gathers its rows on-device with the SWDGE dma_gather instruction,
and the host scatters the gathered rows back to the original positions
(the "all-to-all" of HugeCTR's localized embedding, at unshard time).

Perf model (from the 88us windowed baseline's trace, ntff dma slices):
  - per-engine SWDGE gather throughput is ~15-17ns per descriptor for
    256B..1KB descriptors -- the cost model's 2x penalty for <512B
    descriptors is NOT observed on HW. So window descriptors (which
    added ~28% garbage payload + ~18% padding, all of it also written
    back out) LOSE to one-descriptor-per-unique-row on both directions.
  - GpSimd (Pool) descriptor emission costs ~1.0us fixed per
    dma_gather call + ~0.34ns/desc, serial on the Pool stream. With
    896-idx calls (packet ceiling for single_packet=True) the 25-call
    emission (~30us) rivaled the DMA time. Fix: 2048-idx calls with
    single_packet=False (128 descs/ring, streamed as 2 packets) ->
    3 calls/bin, 12+warmup calls, ~22us emission.
  - calls round-robin the 4 SWDGE queues so all 16 DMA engines stay
    fed; per-bin output writes (fp16, no garbage, ~2.5% padding)
    overlap the remaining gathers on the two HWDGE rings (Sync/Scalar).
  - warmup dma_gather absorbs the ~9us cold SWDGE init during the idx
    loads; fp16 table/output halves both HBM directions (max rel err
    2^-11 vs the 2e-2 gate).

dma_gather layout (probed on HW in an earlier session): indices are
int16, wrapped over 16 partitions (ordinal i reads idx[i % 16, i // 16])
and replicated to all 8 Q7-core partition groups; gathered ordinal i
lands at dst[i % 128, i // 128]; negative idxs at the end generate no
descriptors, but each call keeps >= 16 leading non-negative idxs so all
16 engine rings still fire their completion-semaphore descriptor.
"""

import time

import numpy as np

VOCAB = 1_000_000
BATCH = 8192
SLOTS = 26
VEC = 128
NCORES = 8
NSUB = 4  # bins per core; int16 gather idx needs rows <= 32767
RSUB = VOCAB // (NCORES * NSUB)  # 31250 rows per bin
SHARD = RSUB * NSUB  # 125000 rows per core
P = 128
# Idxs per dma_gather call: 56 data descs + 1 sem desc per engine ring,
# under the 64-descriptor single-packet ceiling. single_packet=True is
# load-bearing: coalesced 56-desc ring packets process at ~14ns/desc;
# single_packet=False (measured, CH=2048) degrades to per-descriptor
# packets at ~32ns/desc -> 83.6us total vs ~21us gather-phase here.
CH = 896

LAST_RUN = None  # BassKernelResults of the most recent device run (for test.py)


def _chunks_of(N: int):
    out = []
    o = 0
    while o < N:
        out.append((o, min(CH, N - o)))
        o += CH
    return out


def _build_program(NCL: int, chunks):
    """One SPMD program for all 8 cores. NCL = padded idx slots per bin
    (multiple of 128, identical across cores/bins).

    Per core:
      shard [SHARD, VEC] fp16   - this core's 4 bins, concatenated
      idx   [P, ICOLS] i16      - [8 warm cols][bin0][bin1][bin2][bin3]
      cnt   [1, NCALL] i32      - per-gather-call runtime num_idxs
      out   [P, NSUB*W] fp16    - W = NCL cols per bin (one fp16 row
                                  per gathered ordinal, dst[i%128,i//128])
    """
    import concourse.bacc as bacc
    from concourse import mybir
    from concourse.library_config import mlp

    ncalls_bin = len(chunks)
    icols_bin = NCL // 16
    W = NCL  # output cols per bin: (NCL//128) groups of VEC
    ICOLS = NSUB * icols_bin
    NCALL = NSUB * ncalls_bin

    nc = bacc.Bacc("TRN2", num_swdge_queues=4)
    shard = nc.declare_dram_parameter(
        "shard", [SHARD, VEC], mybir.dt.float16, isOutput=False
    )
    idx = nc.declare_dram_parameter("idx", [P, ICOLS], mybir.dt.int16, isOutput=False)
    cnt = nc.declare_dram_parameter("cnt", [1, NCALL], mybir.dt.int32, isOutput=False)
    out = nc.declare_dram_parameter(
        "out", [P, NSUB * W], mybir.dt.float16, isOutput=True
    )

    sem_in = nc.alloc_semaphore("sem_in")
    sem_warm = nc.alloc_semaphore("sem_warm")
    sem_wi = nc.alloc_semaphore("sem_wi")
    sem_prep = nc.alloc_semaphore("sem_prep")
    # per-bin gather sems; the last bin's chunks are split across two
    # sems so the final write waits only on the final chunk.
    sem_g = [nc.alloc_semaphore(f"sem_g{s}") for s in range(NSUB + 1)]
    sem_out = nc.alloc_semaphore()

    idx_sb = nc.alloc_sbuf_tensor("idx_sb", [P, ICOLS], mybir.dt.int16).ap()
    warm_idx = nc.alloc_sbuf_tensor("warm_idx", [P, 8], mybir.dt.int16).ap()
    cnt_sb = nc.alloc_sbuf_tensor("cnt_sb", [1, NCALL], mybir.dt.int32).ap()
    warm_out = nc.alloc_sbuf_tensor("warm_out", [P, 1, VEC], mybir.dt.float16).ap()
    g_buf = nc.alloc_sbuf_tensor("g", [P, NSUB * W], mybir.dt.float16).ap()

    nc.gpsimd.load_library(mlp)
    # Warm idx comes from a DVE memset (no input-DMA dependency) so the
    # warmup gather can dispatch as early as possible -- its ~5-6us cold
    # Q7/ucode launch is the long pole of startup.
    nc.vector.memset(warm_idx, 0).then_inc(sem_wi, 1)
    nc.sync.dma_start(out=cnt_sb[:], in_=cnt[:, :]).then_inc(sem_in, 16)
    for s in range(NSUB):
        a, b = s * icols_bin, (s + 1) * icols_bin
        nc.sync.dma_start(out=idx_sb[:, a:b], in_=idx[:, a:b]).then_inc(sem_in, 16)

    warm_reg = nc.gpsimd.to_reg(128)
    cregs = [nc.gpsimd.alloc_register(name=f"creg{t}") for t in range(NCALL)]

    nc.gpsimd.wait_ge(sem_wi, 1)
    nc.gpsimd.dma_gather(
        warm_out[:, :, :],
        shard[0:RSUB, :],
        warm_idx,
        128,
        warm_reg,
        VEC,
        queue_num=0,
    ).then_inc(sem_warm, 16)

    # Batched loads, <= 24 regs each (52-wide measured failing to lower).
    nc.gpsimd.wait_ge(sem_in, 16)
    for i in range(0, NCALL, 24):
        j = min(i + 24, NCALL)
        nc.gpsimd.reg_load(cregs[i:j], cnt_sb[0:1, i:j])

    # Gather calls as PREPARE_ONLY + lagged explicit triggers. A normal
    # (gen_mode=0) call retires only after its transfer completes, and the
    # Pool pipeline holds just 4 calls, so rounds of 4 ran at call-lifetime
    # cadence (emission ~6.5us + drain sync ~1us). Preps retire at
    # emission end; the trigger for call k fires right after prep k's
    # sem (lag 4 = pipeline depth), so the Q7s emit back-to-back and the
    # DMA drain overlaps emission fully. Ring capacity check: 2 calls x
    # 57 descs = 114 <= 128 ring slots per engine per queue.
    qn = 1  # warmup used q0
    t = 0
    trig_q = []  # queue of call t (FIFO)
    ntrig = 0
    for s in range(NSUB):
        nc.gpsimd.wait_ge(sem_in, 16 * (s + 2))
        ibase = s * icols_bin
        for ci, (o, sz) in enumerate(chunks):
            if s < NSUB - 1:
                sem = sem_g[s]
            else:
                sem = sem_g[NSUB - 1] if ci < ncalls_bin - 1 else sem_g[NSUB]
            dst = g_buf[
                :, s * W + (o // 128) * VEC : s * W + ((o + sz) // 128) * VEC
            ].rearrange("p (k e) -> p k e", e=VEC)
            nc.gpsimd.dma_gather(
                dst,
                shard[s * RSUB : (s + 1) * RSUB, :],
                idx_sb[:, ibase + o // 16 : ibase + (o + sz) // 16],
                sz,
                cregs[t],
                VEC,
                single_packet=(sz <= 896),
                prepare_only=True,
                sem=sem,
                queue_num=qn % 4,
            ).then_inc(sem_prep, 1)
            trig_q.append(qn % 4)
            qn += 1
            t += 1
            if t - ntrig > 4:
                nc.gpsimd.wait_ge(sem_prep, ntrig + 1)
                nc.gpsimd.trigger_dma(count=1, queue_num=trig_q[ntrig])
                ntrig += 1
    assert t == NCALL
    while ntrig < NCALL:
        nc.gpsimd.wait_ge(sem_prep, ntrig + 1)
        nc.gpsimd.trigger_dma(count=1, queue_num=trig_q[ntrig])
        ntrig += 1

    # Whole-bin writes alternating between the two HWDGE rings
    # (Sync/Scalar); the last bin is split [chunks 0..n-2 | last chunk]
    # so the tail write waits only on the final gather call.
    last_o = chunks[-1][0]
    writes = []  # (engine_idx, sem, need, col0, col1)
    for s in range(NSUB - 1):
        writes.append((s % 2, sem_g[s], 16 * ncalls_bin, s * W, (s + 1) * W))
    s = NSUB - 1
    writes.append(
        (s % 2, sem_g[s], 16 * (ncalls_bin - 1), s * W, s * W + (last_o // 128) * VEC)
    )
    writes.append((NSUB % 2, sem_g[NSUB], 16, s * W + (last_o // 128) * VEC, NSUB * W))
    for ei, sem, need, c0, c1 in writes:
        eng = nc.sync if ei == 0 else nc.scalar
        eng.wait_ge(sem, need)
        eng.dma_start(out=out[:, c0:c1], in_=g_buf[:, c0:c1]).then_inc(sem_out, 16)
    nc.sync.wait_ge(sem_out, 16 * len(writes))
    nc.sync.wait_ge(sem_warm, 16)
    nc.finalize()
    return nc


def _wrap_cols(vals: np.ndarray, N: int, ecount: int) -> np.ndarray:
    """int16 idx block [16, N//16]: element i at [i%16, i//16]; slots
    [len(vals), ecount) hold 0 (valid row, gathered then ignored), slots
    [ecount, N) hold -1 (skipped by the ucode)."""
    li = np.full(N, -1, np.int16)
    li[:ecount] = 0
    li[: len(vals)] = vals.astype(np.int16)
    return li.reshape(N // 16, 16).T


def _gather_on_device(table_f16: np.ndarray, uniq: np.ndarray) -> np.ndarray:
    """emb[i] = table[uniq[i]] (fp16) computed on 8 NeuronCores."""
    global LAST_RUN
    from concourse.bass_utils import run_bass_kernel_spmd

    total = uniq.shape[0]
    nbins = NCORES * NSUB
    bin_id = (uniq // RSUB).astype(np.int32)
    local = (uniq - bin_id.astype(np.int64) * RSUB).astype(np.int32)
    counts = np.bincount(bin_id, minlength=nbins)
    assert counts.sum() == total
    bin_start = np.concatenate(([0], np.cumsum(counts)))

    NCL = max(P, ((int(counts.max()) + P - 1) // P) * P)
    chunks = _chunks_of(NCL)
    ncalls_bin = len(chunks)
    icols_bin = NCL // 16
    W = NCL

    in_maps = []
    for core in range(NCORES):
        blocks = []
        cvals = []
        for s in range(NSUB):
            b = core * NSUB + s
            n = int(counts[b])
            vals = local[bin_start[b] : bin_start[b + 1]]
            o_last = chunks[-1][0]
            ecount = max(n, o_last + 16)
            blocks.append(_wrap_cols(vals, NCL, ecount))
            for o, sz in chunks:
                cvals.append(min(max(ecount - o, 0), sz))
        in_maps.append(
            {
                "shard": np.ascontiguousarray(
                    table_f16[core * SHARD : (core + 1) * SHARD]
                ),
                "idx": np.ascontiguousarray(
                    np.tile(np.concatenate(blocks, axis=1), (8, 1))
                ),
                "cnt": np.array([cvals], np.int32),
            }
        )

    # The shared device occasionally wedges transiently
    # (NRT_EXEC_UNIT_UNRECOVERABLE / profile-stop rc=-1); a fresh attempt
    # after a short pause recovers it.
    for attempt in range(3):
        try:
            nc = _build_program(NCL, chunks)
            LAST_RUN = run_bass_kernel_spmd(nc, in_maps, list(range(NCORES)))
            break
        except Exception:
            if attempt == 2:
                raise
            time.sleep(10)
    res = LAST_RUN.results

    emb = np.empty((total, VEC), np.float16)
    for core in range(NCORES):
        o = np.asarray(res[core]["out"])
        for s in range(NSUB):
            b = core * NSUB + s
            n = int(counts[b])
            if n == 0:
                continue
            reg = o[:, s * W : (s + 1) * W]
            # gathered ordinal i lands at [i % 128, (i // 128)*VEC : ...]
            rows = (
                reg.reshape(P, NCL // 128, VEC).transpose(1, 0, 2).reshape(NCL, VEC)
            )
            emb[bin_start[b] : bin_start[b + 1]] = rows[:n]
    return emb


def kernel(table, row_offsets, value_tensors, nnz_array=None, output_shape=None):
    table = np.asarray(table, dtype=np.float32)
    assert table.shape == (VOCAB, VEC)
    v = np.asarray(value_tensors).astype(np.int64).ravel()
    total = v.shape[0]

    table_f16 = table.astype(np.float16)
    uniq, inverse = np.unique(v, return_inverse=True)
    emb_u = _gather_on_device(table_f16, uniq)
    emb = emb_u[inverse].astype(np.float32)

    n_rows = BATCH * SLOTS
    ro = np.asarray(row_offsets).astype(np.int64).ravel()
    if total == n_rows and np.array_equal(ro, np.arange(total + 1)):
        return emb.reshape(BATCH, SLOTS, VEC)
    # General CSR fallback (never hit with the reference's arange offsets):
    # sum-combine values per segment on the host.
    seg = np.searchsorted(ro, np.arange(total), side="right") - 1
    combined = np.zeros((n_rows, VEC), np.float32)
    np.add.at(combined, seg, emb)
    return combined.reshape(BATCH, SLOTS, VEC)
